# revision 32
# baseline (speedup 1.0000x reference)
"""NeRF hierarchical sampling + positional encoding kernel for Trainium2.

Full inputs -> shard rays across 8 cores -> Bass kernel per core -> full output.

Sampling is dense-free: with stratified u, su_s = s + ur_s lies in [s, s+1),
so the merge rank pos[k] = #{s: su_s < y_k} collapses to
  pos[k] = floor(y_k) + (ur[floor(y_k)] < y_k - floor(y_k)).
The single ur-gather and the four value gathers (cdf/bins at below/above)
run as per-partition local_scatter (fp32 as int16 pairs, distinct indices
by run-end dedupe) + fill scans on DVE. Gathered values are bit-exact, so
the interpolation matches the fp32 reference.

Encoding: magic-round (ACT Copy, INV_2PI folded into the scale) ->
Cody-Waite range reduction (DVE custom op) -> ACT Sin (cos via +pi/2 bias
on the reduced argument). The loop is software-pipelined one tile deep:
tile t's DVE sampling overlaps tile t-1's ACT sins.
"""

import os
import sys

for _p in ("/opt/trn_rl_repo", "/root/.axon_site/_ro/trn_rl_repo"):
    if os.path.isdir(_p) and _p not in sys.path:
        sys.path.insert(0, _p)

import numpy as np

import concourse.bass as bass
import concourse.bacc as bacc
import concourse.mybir as mybir
from concourse import tile
from concourse import library_config

F32 = mybir.dt.float32
BF16 = mybir.dt.bfloat16
I16 = mybir.dt.int16
OUT_DT = F32              # staging/store dtype
ALU = mybir.AluOpType
ACTF = mybir.ActivationFunctionType

R, N, S = 8192, 128, 128
NCORES = 8
RC = R // NCORES          # rays per core
NT = RC // 128            # ray tiles per core (128 rays each)
DEG = 10
EPS = 1e-5
CH = 120                  # output channels per sample
OUTW = S * CH             # flattened output row per ray

TWO_PI = 6.283185307179586
INV_2PI = float(np.float32(1.0 / TWO_PI))
MAGIC = float(np.float32(1.5 * 2**23))  # round-to-int magic constant
# Cody-Waite split of 2*pi: c1,c2 few-bit exact, c3 fp32 remainder
CW1 = 6.25
CW2 = 0.033203125
CW3 = float(np.float32(TWO_PI - CW1 - CW2))
HALF_PI = float(np.float32(0.5 * np.pi))


def _emit_core_kernel(nc):
    org_h = nc.dram_tensor("origins", [RC, 3], F32, kind="ExternalInput")
    dir_h = nc.dram_tensor("directions", [RC, 3], F32, kind="ExternalInput")
    bins_h = nc.dram_tensor("bins", [RC, N], F32, kind="ExternalInput")
    w_h = nc.dram_tensor("weights", [RC, N], F32, kind="ExternalInput")
    ur_h = nc.dram_tensor("u_rand", [RC, S], F32, kind="ExternalInput")
    out_h = nc.dram_tensor("out", [RC, OUTW], OUT_DT, kind="ExternalOutput")

    with tile.TileContext(nc) as tc:
        with (
            tc.tile_pool(name="io", bufs=2) as io,
            tc.tile_pool(name="stage", bufs=2) as stage_pool,
            tc.tile_pool(name="work", bufs=1) as work,
            tc.tile_pool(name="cross", bufs=2) as cross,
            tc.tile_pool(name="const", bufs=1) as cpool,
        ):
            nc.gpsimd.load_library(library_config.local_scatter)

            # --- constants (once) ---
            ones_t = cpool.tile([128, S], F32)
            nc.vector.memset(ones_t[:, :], 1.0)
            iota_f = cpool.tile([128, S], F32)
            nc.vector.tensor_tensor_scan(iota_f[:, :], ones_t[:, :],
                                         ones_t[:, :], -1.0,
                                         ALU.add, ALU.bypass)
            kp1_i16 = cpool.tile([128, S], I16)
            kp1_f = cpool.tile([128, S], F32)
            nc.vector.tensor_scalar(kp1_f[:, :], iota_f[:, :], 1.0, None,
                                    ALU.add)
            nc.vector.tensor_copy(kp1_i16[:, :], kp1_f[:, :])
            hpi_t = cpool.tile([128, 1], F32)
            nc.vector.memset(hpi_t[:, :], HALF_PI)

            # batched view-encode constants [128, 3*DEG], l-major
            scl_t = cpool.tile([128, 3 * DEG], F32)
            scv_t = cpool.tile([128, 3 * DEG], F32)
            c1_t = cpool.tile([128, 3 * DEG], F32)
            c2_t = cpool.tile([128, 3 * DEG], F32)
            c3_t = cpool.tile([128, 3 * DEG], F32)
            for l in range(DEG):
                sc = float(2.0 ** l)
                sl = slice(3 * l, 3 * l + 3)
                nc.vector.memset(scl_t[:, sl], sc)
                nc.vector.memset(scv_t[:, sl], float(np.float32(INV_2PI) * sc))
                nc.vector.memset(c1_t[:, sl], CW1 / sc)
                nc.vector.memset(c2_t[:, sl], CW2 / sc)
                nc.vector.memset(c3_t[:, sl], CW3 / sc)

            def sample(t, st):
                """Sampling + interpolation + view encode + magic pre-pass.

                Generator: yields at chunk boundaries so the driver can
                interleave emission with the previous tile's encode steps
                (DVE executes in order; interleaving keeps it busy while
                ACT works through the sins).
                """
                r0 = t * 128
                bins_t = io.tile_from(bins_h[r0:r0 + 128, :])
                w_t = io.tile_from(w_h[r0:r0 + 128, :])
                ur_t = io.tile_from(ur_h[r0:r0 + 128, :])
                org_t = io.tile_from(org_h[r0:r0 + 128, :])
                dir_t = io.tile_from(dir_h[r0:r0 + 128, :])

                # ---- view encode, batched over l: vt [128, 60] ----
                dir_b = dir_t[:, :].unsqueeze(1).broadcast_to((128, DEG, 3))
                scv_3 = scv_t[:, :].rearrange("p (l k) -> p l k", k=3)
                c1_3 = c1_t[:, :].rearrange("p (l k) -> p l k", k=3)
                t1v = work.tile([128, DEG * 3], F32, tag="t1v")
                t1v_3 = t1v[:, :].rearrange("p (l k) -> p l k", k=3)
                nc.vector.tensor_tensor(t1v_3, dir_b, scv_3, ALU.mult)
                nc.vector.tensor_scalar(t1v[:, :], t1v[:, :], MAGIC, None,
                                        ALU.add)
                kfv = work.tile([128, DEG * 3], F32, tag="kfv")
                nc.vector.tensor_scalar(kfv[:, :], t1v[:, :], MAGIC, None,
                                        ALU.subtract)
                rsv = work.tile([128, DEG * 3], F32, tag="rsv")
                rsv_3 = rsv[:, :].rearrange("p (l k) -> p l k", k=3)
                kc = work.tile([128, DEG * 3], F32, tag="kc")
                kc_3 = kc[:, :].rearrange("p (l k) -> p l k", k=3)
                nc.vector.tensor_tensor(kc_3, kfv[:, :].rearrange(
                    "p (l k) -> p l k", k=3), c1_3, ALU.mult)
                nc.vector.tensor_tensor(rsv_3, dir_b, kc_3, ALU.subtract)
                nc.vector.tensor_tensor(kc[:, :], kfv[:, :], c2_t[:, :],
                                        ALU.mult)
                nc.vector.tensor_tensor(rsv[:, :], rsv[:, :], kc[:, :],
                                        ALU.subtract)
                nc.vector.tensor_tensor(kc[:, :], kfv[:, :], c3_t[:, :],
                                        ALU.mult)
                nc.vector.tensor_tensor(rsv[:, :], rsv[:, :], kc[:, :],
                                        ALU.subtract)
                rss = cross.tile([128, DEG * 3], F32, tag="rss")
                nc.vector.tensor_tensor(rss[:, :], rsv[:, :], scl_t[:, :],
                                        ALU.mult)

                yield

                # ---- pdf / cdf ----
                wsum = work.tile([128, 1], F32, tag="wsum")
                nc.vector.tensor_reduce(wsum[:, :], w_t[:, 0:N - 1],
                                        axis=mybir.AxisListType.X, op=ALU.add)
                pad = work.tile([128, 1], F32, tag="pad")
                nc.vector.tensor_scalar(pad[:, :], wsum[:, :], -1.0, EPS,
                                        ALU.mult, ALU.add)
                nc.vector.tensor_scalar(pad[:, :], pad[:, :], 0.0, None,
                                        ALU.max)
                wsum2 = work.tile([128, 1], F32, tag="wsum2")
                nc.vector.tensor_tensor(wsum2[:, :], wsum[:, :], pad[:, :],
                                        ALU.add)
                rws = work.tile([128, 1], F32, tag="rws")
                nc.vector.reciprocal(rws[:, :], wsum2[:, :])
                padc = work.tile([128, 1], F32, tag="padc")
                nc.vector.tensor_scalar(padc[:, :], pad[:, :], 1.0 / (N - 1),
                                        None, ALU.mult)
                pdf = work.tile([128, N - 1], F32, tag="pdf")
                nc.vector.tensor_scalar(pdf[:, :], w_t[:, 0:N - 1],
                                        padc[:, 0:1], None, ALU.add)
                nc.vector.tensor_scalar(pdf[:, :], pdf[:, :], rws[:, 0:1],
                                        None, ALU.mult)

                yield
                cdf = work.tile([128, N], F32, tag="cdf")
                nc.vector.memset(cdf[:, 0:1], 0.0)
                nc.vector.memset(cdf[:, N - 1:N], 1.0)
                cs = work.tile([128, N - 2], F32, tag="cs")
                nc.vector.tensor_tensor_scan(cs[:, :], ones_t[:, 0:N - 2],
                                             pdf[:, 0:N - 2], 0.0,
                                             ALU.mult, ALU.add)
                nc.vector.tensor_scalar(cdf[:, 1:N - 1], cs[:, :], 1.0, None,
                                        ALU.min)

                yield
                # ---- pos[k] = fk + (ur[fk] < frac) ----
                y = work.tile([128, N], F32, tag="y")
                nc.vector.tensor_scalar(y[:, :], cdf[:, :], float(S), None,
                                        ALU.mult)
                ym = work.tile([128, N], F32, tag="ym")
                nc.vector.tensor_scalar(ym[:, :], y[:, :], 0.5, None, ALU.max)
                tmag = work.tile([128, N], F32, tag="tmag")
                nc.vector.tensor_scalar(tmag[:, :], ym[:, :], -0.5, MAGIC,
                                        ALU.add, ALU.add)
                fk = work.tile([128, N], F32, tag="fk")
                nc.vector.tensor_scalar(fk[:, :], tmag[:, :], MAGIC, None,
                                        ALU.subtract)
                frac = work.tile([128, N], F32, tag="frac")
                nc.vector.tensor_tensor(frac[:, :], y[:, :], fk[:, :],
                                        ALU.subtract)
                fkc = work.tile([128, N], F32, tag="fkc")
                nc.vector.tensor_scalar(fkc[:, :], fk[:, :], float(N - 1),
                                        None, ALU.min)

                yield
                # A[v] = #{k: fkc[k] <= v}: dedupe + scatter k+1 + maxscan
                fknext = work.tile([128, N], F32, tag="fknext")
                nc.vector.memset(fknext[:, N - 1:N], 500.0)
                nc.vector.tensor_copy(fknext[:, 0:N - 1], fkc[:, 1:N])
                bA = work.tile([128, N], F32, tag="bA")
                nc.vector.tensor_tensor(bA[:, :], fkc[:, :], fknext[:, :],
                                        ALU.is_lt)
                idxAf = work.tile([128, N], F32, tag="idxAf")
                nc.vector.tensor_tensor(idxAf[:, :], fkc[:, :], bA[:, :],
                                        ALU.mult)
                nc.vector.tensor_tensor(idxAf[:, :], idxAf[:, :], bA[:, :],
                                        ALU.add)
                nc.vector.tensor_scalar(idxAf[:, :], idxAf[:, :], -1.0, None,
                                        ALU.add)
                idxA = work.tile([128, N], I16, tag="idxA")
                nc.vector.tensor_copy(idxA[:, :], idxAf[:, :])
                A_sc = work.tile([128, S], I16, tag="A_sc")
                nc.gpsimd.local_scatter(A_sc[:, :], kp1_i16[:, :], idxA[:, :],
                                        channels=128, num_elems=S, num_idxs=N)
                A_f = work.tile([128, S], F32, tag="A_f")
                nc.vector.tensor_copy(A_f[:, :], A_sc[:, :])
                A = work.tile([128, S], F32, tag="A")
                nc.vector.tensor_tensor_scan(A[:, :], A_f[:, :], A_f[:, :],
                                             0.0, ALU.max, ALU.bypass)

                yield
                # ur scatter to first-k slots
                a_s = work.tile([128, S], F32, tag="a_s")
                nc.vector.memset(a_s[:, 0:1], 0.0)
                nc.vector.tensor_copy(a_s[:, 1:S], A[:, 0:S - 1])
                bw = work.tile([128, S], F32, tag="bw")
                nc.vector.tensor_tensor(bw[:, :], a_s[:, :], A[:, :],
                                        ALU.is_lt)
                evu = work.tile([128, S], F32, tag="evu")
                nc.vector.tensor_scalar(evu[:, :], a_s[:, :], 2.0, None,
                                        ALU.mult)
                nc.vector.tensor_tensor(evu[:, :], evu[:, :], bw[:, :],
                                        ALU.mult)
                bw2 = work.tile([128, S], F32, tag="bw2")
                nc.vector.tensor_scalar(bw2[:, :], bw[:, :], 2.0, -2.0,
                                        ALU.mult, ALU.add)
                nc.vector.tensor_tensor(evu[:, :], evu[:, :], bw2[:, :],
                                        ALU.add)
                odu = work.tile([128, S], F32, tag="odu")
                nc.vector.tensor_scalar(odu[:, :], evu[:, :], 1.0, None,
                                        ALU.add)
                idxU = work.tile([128, 2 * S], I16, tag="idxU")
                idxUv = idxU[:, :].rearrange("p (k two) -> p k two", two=2)
                nc.vector.tensor_copy(idxUv[:, :, 0], evu[:, :])
                nc.vector.tensor_copy(idxUv[:, :, 1], odu[:, :])
                U_scf = work.tile([128, S], F32, tag="U_scf")
                nc.gpsimd.local_scatter(U_scf[:, :].bitcast(I16),
                                        ur_t[:, :].bitcast(I16), idxU[:, :],
                                        channels=128, num_elems=2 * S,
                                        num_idxs=2 * S)
                yield
                fkprev = work.tile([128, N], F32, tag="fkprev")
                nc.vector.memset(fkprev[:, 0:1], -1.0)
                nc.vector.tensor_copy(fkprev[:, 1:N], fkc[:, 0:N - 1])
                mc = work.tile([128, N], F32, tag="mc")
                nc.vector.tensor_tensor(mc[:, :], fkprev[:, :], fkc[:, :],
                                        ALU.is_ge)
                r_ur = work.tile([128, N], F32, tag="r_ur")
                nc.vector.tensor_tensor_scan(r_ur[:, :], mc[:, :],
                                             U_scf[:, :], 0.0,
                                             ALU.mult, ALU.add)
                ind = work.tile([128, N], F32, tag="ind")
                nc.vector.tensor_tensor(ind[:, :], r_ur[:, :], frac[:, :],
                                        ALU.is_lt)
                pos = work.tile([128, N], F32, tag="pos")
                nc.vector.tensor_tensor(pos[:, :], fk[:, :], ind[:, :],
                                        ALU.add)

                yield
                # ---- value scatters: v[k] -> slot pos[k], dedup by run ----
                P = work.tile([128, N], F32, tag="P")
                nc.vector.memset(P[:, N - 1:N], float(S))
                nc.vector.tensor_copy(P[:, 0:N - 1], pos[:, 1:N])
                bv = work.tile([128, N], F32, tag="bv")
                nc.vector.tensor_tensor(bv[:, :], pos[:, :], P[:, :],
                                        ALU.is_lt)
                mok = work.tile([128, N], F32, tag="mok")
                nc.vector.tensor_scalar(mok[:, :], pos[:, :], float(S - 1),
                                        None, ALU.is_le)
                mm = work.tile([128, N], F32, tag="mm")
                nc.vector.tensor_tensor(mm[:, :], bv[:, :], mok[:, :],
                                        ALU.mult)
                ev = work.tile([128, N], F32, tag="ev")
                nc.vector.tensor_scalar(ev[:, :], pos[:, :], 2.0, None,
                                        ALU.mult)
                nc.vector.tensor_tensor(ev[:, :], ev[:, :], mm[:, :],
                                        ALU.mult)
                mm2 = work.tile([128, N], F32, tag="mm2")
                nc.vector.tensor_scalar(mm2[:, :], mm[:, :], 2.0, -2.0,
                                        ALU.mult, ALU.add)
                nc.vector.tensor_tensor(ev[:, :], ev[:, :], mm2[:, :],
                                        ALU.add)
                od = work.tile([128, N], F32, tag="od")
                nc.vector.tensor_scalar(od[:, :], ev[:, :], 1.0, None,
                                        ALU.add)
                idx2 = work.tile([128, 2 * N], I16, tag="idx2")
                idx2v = idx2[:, :].rearrange("p (k two) -> p k two", two=2)
                nc.vector.tensor_copy(idx2v[:, :, 0], ev[:, :])
                nc.vector.tensor_copy(idx2v[:, :, 1], od[:, :])

                yield
                cdf1 = work.tile([128, N], F32, tag="cdf1")
                nc.vector.tensor_copy(cdf1[:, 0:N - 1], cdf[:, 1:N])
                nc.vector.memset(cdf1[:, N - 1:N], 1.0)
                bins1 = work.tile([128, N], F32, tag="bins1")
                nc.vector.tensor_copy(bins1[:, 0:N - 1], bins_t[:, 1:N])
                nc.vector.memset(bins1[:, N - 1:N], 0.0)

                outs = {}
                for name, src in (("cdf_g0", cdf), ("cdf_g1", cdf1),
                                  ("bins_g0", bins_t), ("bins_g1", bins1)):
                    sc_t = work.tile([128, N], F32, tag=f"sc_{name}")
                    nc.gpsimd.local_scatter(
                        sc_t[:, :].bitcast(I16), src[:, :].bitcast(I16),
                        idx2[:, :], channels=128, num_elems=2 * N,
                        num_idxs=2 * N)
                    g = work.tile([128, S], F32, tag=f"g_{name}")
                    nc.vector.tensor_tensor_scan(g[:, :], sc_t[:, :],
                                                 sc_t[:, :], 0.0,
                                                 ALU.max, ALU.bypass)
                    outs[name] = g

                yield
                # ---- u + interpolation ----
                su = work.tile([128, S], F32, tag="su")
                nc.vector.tensor_tensor(su[:, :], iota_f[:, :], ur_t[:, :],
                                        ALU.add)
                u = work.tile([128, S], F32, tag="u")
                nc.vector.tensor_scalar(u[:, :], su[:, :], 1.0 / S, None,
                                        ALU.mult)
                denom = work.tile([128, S], F32, tag="denom")
                nc.vector.tensor_tensor(denom[:, :], outs["cdf_g1"][:, :],
                                        outs["cdf_g0"][:, :], ALU.subtract)
                mask = work.tile([128, S], F32, tag="mask")
                nc.vector.tensor_scalar(mask[:, :], denom[:, :], EPS, None,
                                        ALU.is_lt)
                omd = work.tile([128, S], F32, tag="omd")
                nc.vector.tensor_scalar(omd[:, :], denom[:, :], -1.0, 1.0,
                                        ALU.mult, ALU.add)
                nc.vector.tensor_tensor(omd[:, :], mask[:, :], omd[:, :],
                                        ALU.mult)
                denom2 = work.tile([128, S], F32, tag="denom2")
                nc.vector.tensor_tensor(denom2[:, :], denom[:, :], omd[:, :],
                                        ALU.add)
                yield
                rcp = work.tile([128, S], F32, tag="rcp")
                nc.vector.reciprocal(rcp[:, :], denom2[:, :])
                tt = work.tile([128, S], F32, tag="tt")
                nc.vector.tensor_tensor(tt[:, :], u[:, :],
                                        outs["cdf_g0"][:, :], ALU.subtract)
                nc.vector.tensor_tensor(tt[:, :], tt[:, :], rcp[:, :],
                                        ALU.mult)
                db = work.tile([128, S], F32, tag="db")
                nc.vector.tensor_tensor(db[:, :], outs["bins_g1"][:, :],
                                        outs["bins_g0"][:, :], ALU.subtract)
                smp = work.tile([128, S], F32, tag="smp")
                nc.vector.tensor_tensor(smp[:, :], tt[:, :], db[:, :],
                                        ALU.mult)
                nc.vector.tensor_tensor(smp[:, :], smp[:, :],
                                        outs["bins_g0"][:, :], ALU.add)

                yield
                # ---- points, coord-major [128, 3*S] ----
                pts = cross.tile([128, 3 * S], F32, tag="pts")
                for k in range(3):
                    nc.vector.scalar_tensor_tensor(
                        pts[:, k * S:(k + 1) * S], smp[:, :],
                        dir_t[:, k:k + 1],
                        org_t[:, k:k + 1].broadcast_to((128, S)),
                        ALU.mult, ALU.add)

                # magic-round pre-pass on ACT: t1_l = pts*(2^l/2pi) + MAGIC
                t1a = cross.tile([128, DEG * 3 * S], F32, tag="t1a")
                for l in range(DEG):
                    scp = float(np.float32(INV_2PI) * (2.0 ** l))
                    nc.scalar.activation(t1a[:, l * 3 * S:(l + 1) * 3 * S],
                                         pts[:, :], ACTF.Copy,
                                         bias=MAGIC, scale=scp)


                st.update(r0=r0, pts=pts, t1a=t1a, rss=rss)

            def encode(st):
                """Cody-Waite + sins into full-S staging + store.

                Generator: yields after each frequency.
                """
                r0, pts, t1a = st["r0"], st["pts"], st["t1a"]
                stg = stage_pool.tile([128, S * CH], OUT_DT, name="stg",
                                      tag="stg")
                stg3 = stg[:, :].rearrange("p (s c) -> p s c", c=CH)
                rss_b = st["rss"][:, :].unsqueeze(1).broadcast_to(
                    (128, S, 30))
                nc.scalar.activation(stg3[:, :, 60:90], rss_b, ACTF.Sin,
                                     bias=0.0, scale=1.0)
                nc.scalar.activation(stg3[:, :, 90:120], rss_b, ACTF.Sin,
                                     bias=hpi_t[:, 0:1], scale=1.0)
                yield
                kf = work.tile([128, 3 * S], F32, tag="kf")
                rs2 = [work.tile([128, 3 * S], F32, name=f"rs{par}",
                                 tag=f"rs{par}") for par in range(2)]
                for l in range(DEG):
                    sc = float(2.0 ** l)
                    rs = rs2[l % 2]
                    rs_3 = rs[:, :].rearrange("p (k s) -> p k s", k=3)
                    nc.vector.tensor_scalar(kf[:, :],
                                            t1a[:, l * 3 * S:(l + 1) * 3 * S],
                                            MAGIC, None, ALU.subtract)
                    nc.vector.cody_waite_cascade(rs[:, :], pts[:, :],
                                                 kf[:, :], CW1 / sc,
                                                 CW2 / sc, CW3 / sc)
                    sin_dst = stg3[:, :, 3 * l:3 * l + 3].rearrange(
                        "p s c -> p c s")
                    nc.scalar.activation(sin_dst, rs_3, ACTF.Sin,
                                         bias=0.0, scale=sc)
                    cos_dst = stg3[:, :, 30 + 3 * l:30 + 3 * l + 3].rearrange(
                        "p s c -> p c s")
                    nc.scalar.activation(cos_dst, rs_3, ACTF.Sin,
                                         bias=hpi_t[:, 0:1], scale=sc)
                    yield
                nc.sync.dma_start(out_h[r0:r0 + 128, :], stg[:, :])

            def drain(gens):
                done = [False] * len(gens)
                while not all(done):
                    for i, g in enumerate(gens):
                        if not done[i]:
                            try:
                                next(g)
                            except StopIteration:
                                done[i] = True

            prev = None
            for t in range(NT):
                st = {}
                gens = [sample(t, st)]
                if prev is not None:
                    gens.insert(0, encode(prev))
                drain(gens)
                prev = st
            drain([encode(prev)])
    return nc


_NC_CACHE = {}


def _get_nc():
    if "nc" not in _NC_CACHE:
        nc = bacc.Bacc('TRN2', target_bir_lowering=False)
        _emit_core_kernel(nc)
        nc.compile()
        _NC_CACHE["nc"] = nc
    return _NC_CACHE["nc"]


def _shard(inputs):
    in_maps = []
    for c in range(NCORES):
        sl = slice(c * RC, (c + 1) * RC)
        in_maps.append({
            "origins": np.ascontiguousarray(inputs["origins"][sl]),
            "directions": np.ascontiguousarray(inputs["directions"][sl]),
            "bins": np.ascontiguousarray(inputs["bins"][sl]),
            "weights": np.ascontiguousarray(inputs["weights"][sl]),
            "u_rand": np.ascontiguousarray(inputs["u_rand"][sl]),
        })
    return in_maps


def kernel(**inputs):
    from concourse.bass_utils import run_bass_kernel_spmd
    nc = _get_nc()
    in_maps = _shard(inputs)
    res = run_bass_kernel_spmd(nc, in_maps, core_ids=list(range(NCORES)))
    parts = [res.results[c]["out"].reshape(RC, S, CH) for c in range(NCORES)]
    return np.concatenate(parts, axis=0).astype(np.float32)


def simulate_one_core(core_inputs):
    """CoreSim path for numerics debugging (no hardware)."""
    from concourse.bass_interp import CoreSim
    nc = bacc.Bacc('TRN2', target_bir_lowering=False)
    _emit_core_kernel(nc)
    nc.compile()
    sim = CoreSim(nc, require_finite=False, require_nnan=False)
    if sim.instruction_executor is not None:
        sim.instruction_executor.ignore_data_errors = True
    for k, v in core_inputs.items():
        sim.tensor(k)[:] = v
    sim.simulate()
    return np.array(sim.tensor("out")).reshape(RC, S, CH)
